# revision 61
# baseline (speedup 1.0000x reference)
"""AttentionWithRope Trainium2 Bass kernel (v8).

Sharding: 8 cores = 2 batches x 4 head-groups (4 heads / 256 features each).
Each core computes q/k/v projections for its feature slice on its batch,
RoPE, causal attention for its 4 heads, and a partial output projection.
The host sums the 4 partials per batch and adds wo_b.

v8 over v6 (242us -> ~239us, hardware power-state noise is +/-5%):
  - all DRAM inputs pre-laid-out host-side so every DMA is contiguous per
    partition (>=2KB descriptors, full HBM bandwidth; startup shrinks);
    first x chunk split across two queues.
  - rope: one DVE pass folds bias and casts psum->bf16, ACT does the four
    32-partition swap copies (only ACT does partition-crossing copies
    cheaply), cos/sin muls and final add run as 2x-rate bf16 DVE ops
    (rope tables shipped bf16).
  - causal mask folded in BEFORE exp as a -1e30 triangle add on the scores
    PSUM, so exp emits zeros in the masked region and the pv matmuls have
    no post-exp mask dependency (pv is pure PE filler).
  - qhatT/khatT/znT split into per-chunk tiles so phase B's first scores
    depend only on chunk-0 rope writes, not all of phase A (the tile
    framework tracks dependencies per tile).
  - softmax normalization multiplies straight out of PSUM (no zc copy);
    reciprocals batched per head-pair; 1/sums partition-broadcast via a
    DRAM bounce (gpsimd DMA round trip - measured faster than the Pool
    partition_broadcast custom op).
  - scheduling: j=0/j=1 groups' scores emitted back-to-back as one
    super-group so their ~8us serial chains (exp->pv->evac->recip->
    bounce->mul) overlap; half of j=2's phase C held back as ready PE
    filler ahead of the final group's chain-dependent blocks (the PE
    queue is in-order: a stalled matmul blocks everything behind it).

Rejected by measurement: fp8 DoubleRow projections (3.6e-2 rel err AND
slower), tri/copies on the Pool engine (strided/partition-crossing ops
run ~3x slower there), gpsimd.partition_broadcast (1.2us each), holding
back all of phase C for the tail, descending-j group order.
"""

import numpy as np
from contextlib import ExitStack

DIM, HEADS, HD = 1024, 16, 64
B, S = 2, 2048
NC = 8
HPC = 4          # heads per core
F = HPC * HD     # 256 features per core
CH = 512         # token chunk
ROPE_BASE = 10000.0


def _rope_tables():
    theta = ROPE_BASE ** (-np.arange(0, HD, 2, dtype=np.float32) / HD)  # [32]
    pos = np.arange(S, dtype=np.float32)
    ang = pos[:, None] * theta[None, :]          # [S, 32]
    cos, sin = np.cos(ang).T, np.sin(ang).T      # [32, S]
    CT = np.concatenate([cos, cos, cos, cos], 0).astype(np.float32)      # [128, S]
    SST = np.concatenate([-sin, sin, -sin, sin], 0).astype(np.float32)   # [128, S]
    return CT, SST


def _build_program():
    import concourse.bass as bass
    import concourse.mybir as mybir
    import concourse.tile as tile
    from concourse import bacc

    fp32 = mybir.dt.float32
    bf16 = mybir.dt.bfloat16
    fp8 = mybir.dt.float8e4
    AF = mybir.ActivationFunctionType
    ALU = mybir.AluOpType
    DR = mybir.MatmulPerfMode.DoubleRow
    LN32 = float(np.log(32.0))

    nc = bacc.Bacc("TRN2", target_bir_lowering=False, num_devices=NC)

    from bass_rust import add_dep_helper as _adh
    _prev_mm = [None]

    def MM(*args, **kw):
        bi = nc.tensor.matmul(*args, **kw)
        if _prev_mm[0] is not None:
            _adh(bi.ins, _prev_mm[0].ins, sync=False, reason="pe-order")
        _prev_mm[0] = bi
        return bi

    # ---- DRAM I/O (all pre-laid-out host-side, partition-contiguous) ----
    xT_d = nc.dram_tensor("xT", [128, 4 * 8 * CH], bf16, kind="ExternalInput").ap()
    wqT_d = nc.dram_tensor("wqT", [128, 8 * F], bf16, kind="ExternalInput").ap()
    wkT_d = nc.dram_tensor("wkT", [128, 8 * F], bf16, kind="ExternalInput").ap()
    wvT_d = nc.dram_tensor("wvT", [128, 8 * 272], bf16, kind="ExternalInput").ap()
    woT_d = nc.dram_tensor("woT", [128, 2 * DIM], bf16, kind="ExternalInput").ap()
    qb_d = nc.dram_tensor("qb", [128, 2], fp32, kind="ExternalInput").ap()
    kb_d = nc.dram_tensor("kb", [128, 2], fp32, kind="ExternalInput").ap()
    vb_d = nc.dram_tensor("vb", [1, 272], fp32, kind="ExternalInput").ap()
    mask_d = nc.dram_tensor("maskv", [128, 32], fp32, kind="ExternalInput").ap()
    RT_d = nc.dram_tensor("RT", [128, 4 * 2 * CH], bf16, kind="ExternalInput").ap()
    tri_d = nc.dram_tensor("tri", [128, 128], fp32, kind="ExternalInput").ap()
    out_d = nc.dram_tensor("outp", [S, DIM], bf16, kind="ExternalOutput").ap()

    with tile.TileContext(nc) as tc, ExitStack() as ctx:
        # ---------- persistent SBUF ----------
        const = ctx.enter_context(tc.tile_pool(name="const", bufs=1))
        qk_pool = ctx.enter_context(tc.tile_pool(name="qk", bufs=1))
        v_pool = ctx.enter_context(tc.tile_pool(name="v", bufs=1))
        zn_pool = ctx.enter_context(tc.tile_pool(name="zn", bufs=1))

        tri_s = const.tile([128, 128], fp32, tag="tri", name="tri")
        qb_s = const.tile([128, 2], fp32, tag="qb", name="qb")
        kb_s = const.tile([128, 2], fp32, tag="kb", name="kb")
        vb_s = const.tile([128, 272], fp32, tag="vb", name="vb")
        mask_s = const.tile([128, 32], fp32, tag="maskv", name="maskv")
        nl8_s = const.tile([128, 1], fp32, tag="nl8", name="nl8")
        nc.vector.memset(nl8_s[:], -LN32)
        wo_big = const.tile([128, 2 * DIM], bf16, tag="wobig", name="wobig")
        woT_s = [wo_big[:, DIM * t:DIM * t + DIM] for t in range(2)]

        qhatT = [[qk_pool.tile([128, CH], bf16, tag=f"qhatT{t}_{jc}",
                          name=f"qhatT{t}_{jc}") for jc in range(4)]
                 for t in range(2)]
        khatT = [[qk_pool.tile([128, CH], bf16, tag=f"khatT{t}_{jc}",
                          name=f"khatT{t}_{jc}") for jc in range(4)]
                 for t in range(2)]
        # v layout [p, head, ktile, 65]: per-head k-tile pairs contiguous so
        # the dual-fp8 DoubleRow LDWEIGHTS sees a (k, m) block
        v_big = v_pool.tile([128, 4 * 16 * 80], fp8, tag="vbig", name="vbig")
        v_4d = v_big[:].rearrange("p (h i f) -> p h i f", h=4, i=16)
        nc.vector.memset(v_big[:], 0.0)
        vb_big = v_pool.tile([128, 4 * 16 * 65], bf16, tag="vbbig", name="vbbig")
        vb_4d = vb_big[:].rearrange("p (h i f) -> p h i f", h=4, i=16)
        znT = [[zn_pool.tile([128, CH], bf16, tag=f"znT{t}_{jc}",
                        name=f"znT{t}_{jc}") for jc in range(4)]
               for t in range(2)]

        # ---------- phase A: projections + rope, x streamed by token chunk --
        # producer (matmul) / consumer (evac+rope) stages are emitted one
        # stage apart so the in-order ACT/DVE queues never wait at head on
        # an unfinished PSUM producer.
        with tc.tile_pool(name="wqk", bufs=1) as wp, \
             tc.tile_pool(name="xT", bufs=2) as xp, \
             tc.tile_pool(name="ppsum", bufs=3, space="PSUM") as pp, \
             tc.tile_pool(name="ropetmp", bufs=5) as rp:

            wq_big = wp.tile([128, 8 * F], bf16, tag="wqbig", name="wqbig")
            wk_big = wp.tile([128, 8 * F], bf16, tag="wkbig", name="wkbig")
            wv_big = wp.tile([128, 8 * 272], bf16, tag="wvbig", name="wvbig")
            wq_s = [wq_big[:, F * d:F * d + F] for d in range(8)]
            wk_s = [wk_big[:, F * d:F * d + F] for d in range(8)]
            wv_s = [wv_big[:, 272 * d:272 * d + 272] for d in range(8)]

            stages = []

            def qk_stage(x_s, w_s, b_s, dst, fc, ctb, sstb, cs):
                ps = [None]

                def produce():
                    ps[0] = pp.tile([128, CH], fp32, tag="proj", name="proj")
                    for d in range(8):
                        MM(ps[0][:], w_s[d][:, 128 * fc:128 * fc + 128],
                           x_s[d][:], start=(d == 0), stop=(d == 7))

                def consume():
                    # pc = bf16(ps + bias); swaps on ACT; muls/add 2x bf16 DVE
                    pc = rp.tile([128, CH], bf16, tag="pc", name="pc")
                    nc.vector.tensor_scalar_add(pc[:], ps[0][:],
                                                b_s[:, fc:fc + 1])
                    sw = rp.tile([128, CH], bf16, tag="sw", name="sw")
                    # partition-swap copies: ACT only — DVE/Pool take ~1.9us
                    # for partition-crossing copies (slow shuffle path)
                    for o, so in ((0, 32), (32, 0), (64, 96), (96, 64)):
                        nc.scalar.activation(sw[o:o + 32, :],
                                             pc[so:so + 32, :], AF.Identity)
                    qct = rp.tile([128, CH], bf16, tag="qct", name="qct")
                    nc.vector.tensor_mul(qct[:], pc[:], ctb)
                    nc.vector.tensor_mul(sw[:], sw[:], sstb)
                    nc.vector.tensor_add(dst[fc][:], qct[:], sw[:])
                return produce, consume

            def v_stage(x_s, t):
                ps = [None]

                def produce():
                    ps[0] = pp.tile([128, 272], fp32, tag="vproj",
                                    name="vproj")
                    for d in range(8):
                        MM(ps[0][:], x_s[d][:, 128 * (t % 4):128 * (t % 4) + 128],
                           wv_s[d][:], start=(d == 0), stop=(d == 7))

                def consume():
                    vt = rp.tile([128, 272], fp32, tag="vt", name="vt")
                    nc.vector.tensor_add(vt[:], ps[0][:], vb_s[:])
                    vt3 = vt[:].rearrange("p (h f) -> p h f", h=4)
                    nc.vector.tensor_scalar_mul(v_4d[:, :, t, 0:68], vt3,
                                                mask_s[:, 2 * t:2 * t + 1])
                    nc.vector.tensor_scalar_mul(vb_4d[:, :, t, :],
                                                vt3[:, :, 0:65],
                                                mask_s[:, 2 * t + 1:2 * t + 2])
                return produce, consume

            for j in range(4):
                cs = slice(CH * j, CH * j + CH)
                xc = xp.tile([128, 8 * CH], bf16, tag="xc", name="xc")
                x_s = [xc[:, CH * d:CH * d + CH] for d in range(8)]
                if j == 0:
                    # split the first chunk across two queues: halves the
                    # DMA latency gating the very first matmul
                    nc.scalar.dma_start(xc[:, 0:4 * CH],
                                        xT_d[:, 0:4 * CH])
                    nc.gpsimd.dma_start(xc[:, 4 * CH:8 * CH],
                                        xT_d[:, 4 * CH:8 * CH])
                else:
                    nc.sync.dma_start(
                        xc[:], xT_d[:, 8 * CH * j:8 * CH * j + 8 * CH])
                rt = xp.tile([128, 2 * CH], bf16, tag="rt", name="rt")
                ctb, sstb = rt[:, 0:CH], rt[:, CH:2 * CH]
                (nc.gpsimd if j == 0 else nc.sync).dma_start(
                    rt[:], RT_d[:, 2 * CH * j:2 * CH * j + 2 * CH])
                if j == 0:
                    nc.gpsimd.dma_start(qb_s[:], qb_d[:])
                    nc.gpsimd.dma_start(kb_s[:], kb_d[:])
                    nc.sync.dma_start(wq_big[:], wqT_d[:])
                    nc.sync.dma_start(wk_big[:], wkT_d[:])
                    nc.scalar.dma_start(wv_big[:], wvT_d[:])
                    nc.gpsimd.dma_start(vb_s[:],
                                        vb_d[0:1, :].to_broadcast((128, 272)))
                    nc.gpsimd.dma_start(mask_s[:], mask_d[:])
                    nc.gpsimd.dma_start(tri_s[:], tri_d[:])
                if j == 1:
                    nc.sync.dma_start(wo_big[:], woT_d[:])

                for w_s, b_s, dstj in ((wq_s, qb_s, qhatT),
                                       (wk_s, kb_s, khatT)):
                    for fc in range(2):
                        stages.append(qk_stage(x_s, w_s, b_s,
                                               [dstj[0][j], dstj[1][j]], fc,
                                               ctb, sstb, cs))
                for tt in range(4):
                    stages.append(v_stage(x_s, 4 * j + tt))
                # emit this chunk's stages one step out of phase
                while len(stages) > 1:
                    prod, cons = stages.pop(0)
                    prod()
                    stages[0][0]()
                    # swap so the pending producer isn't re-run
                    stages[0] = (lambda: None, stages[0][1])
                    cons()
            prod, cons = stages.pop(0)
            prod()
            cons()

        # ---------- phase B + C: software-pipelined attention ----------
        # every PSUM consumer runs as its own queued block, popped a few
        # score-slots after its producer, so no engine queue blocks at head.
        with tc.tile_pool(name="spsum", bufs=3, space="PSUM") as sp, \
             tc.tile_pool(name="zpsum", bufs=3, space="PSUM") as zp, \
             tc.tile_pool(name="opsum", bufs=2, space="PSUM") as op, \
             tc.tile_pool(name="pT", bufs=1) as ptp, \
             tc.tile_pool(name="sums", bufs=4) as smp, \
             tc.tile_pool(name="rbc", bufs=4) as rbp, \
             tc.tile_pool(name="rdram", bufs=4, space="DRAM") as rdp, \
             tc.tile_pool(name="osbuf", bufs=4) as ob:

            pts = {}

            def s_pair(h, ip, j):
                tH, rH = h // 2, 64 * (h % 2)
                diag = 2 * ip + 1 - 4 * j >= 1
                pb = ptp.tile([128, 1024], bf16 if diag else fp8,
                              tag=f"p{h}_{ip}_{int(diag)}", name="p")
                pts[(h, ip, j)] = pb
                if not diag:
                    pbi = pb[:].rearrange("p (n k) -> p k n", k=2)
                # single-bank score tiles (bufs=4): the PE's psum-reuse wait
                # reaches 4 tiles back (~3 exps of slack) instead of 2, so
                # the matmul stream no longer stalls on each pair's exp;
                # per-tile exps also trim the diagonal overhang exactly
                for ii in range(2):
                    i = 2 * ip + ii
                    d = i - 4 * j
                    c0 = max(0, 128 * d)
                    sps = sp.tile([128, 512], fp32, tag="s", name="s")
                    MM(sps[:, c0:512],
                       khatT[tH][i // 4][rH:rH + 64,
                                         128 * (i % 4):128 * (i % 4) + 128],
                       qhatT[tH][j][rH:rH + 64, c0:CH],
                       start=True, stop=True)
                    if diag and d >= 0:
                        # causal mask folded in BEFORE exp: -1e30 on the
                        # strict lower triangle of the diagonal 128-block
                        nc.vector.tensor_add(sps[:, c0:c0 + 128],
                                             sps[:, c0:c0 + 128],
                                             tri_s[:])
                    # off-diag: p/32 in fp8 (overflow-safe; the 8x in fp8 v
                    # makes the off-diag contribution p*v/4, matched by the
                    # 1/4-scaled bf16 diag path; normalization cancels it);
                    # diag: plain exp in bf16 (row sums can't underflow)
                    if diag:
                        nc.scalar.activation(
                            pb[:, 512 * ii + c0:512 * ii + 512],
                            sps[:, c0:512], AF.Exp)
                    else:
                        nc.scalar.activation(pbi[:, ii, :], sps[:, 0:512],
                                             AF.Exp, bias=nl8_s[:, 0:1])

            def pv_off(h, j, st):
                # off-diagonal DR matmuls: no tri-mask dependency, their exp
                # finished long ago -> pure PE filler
                def go():
                    if j == 0:
                        return
                    zps = zp.tile([80, CH], fp32, tag="z", name="z")
                    st["zps"] = zps
                    for ip in range(2 * j):
                        pbv = pts[(h, ip, j)][:].rearrange(
                            "p (n k) -> p k n", k=2)
                        MM(zps[:, 0:CH],
                           v_4d[:, h, 2 * ip:2 * ip + 2, :],
                           pbv[:, :, 0:CH],
                           start=(ip == 0), stop=False,
                           perf_mode=DR)
                return go

            def pv_diag(h, j, st):
                def go():
                    if j == 0:
                        zps = zp.tile([80, CH], fp32, tag="z", name="z")
                        st["zps"] = zps
                    zps = st["zps"]
                    first = (j == 0)
                    for k, ip in enumerate((2 * j, 2 * j + 1)):
                        for ii in range(2):
                            i = 2 * ip + ii
                            c0 = 128 * (i - 4 * j)
                            MM(zps[0:65, c0:CH],
                               vb_4d[:, h, i, :],
                               pts[(h, ip, j)][:, 512 * ii + c0:512 * ii + 512],
                               start=(first and k == 0 and ii == 0),
                               stop=(k == 1 and ii == 1))
                return go

            def evac_block(h, j, st, srec):
                def go():
                    c = CH * (h % 2)
                    nc.vector.tensor_copy(srec[0:1, c:c + CH],
                                          st["zps"][64:65, :])
                    if c:
                        # both heads' sums in one custom-DVE recip call
                        nc.vector.reciprocal_approx_fast(srec[:], srec[:])
                        rd = rdp.tile([1, 2 * CH], fp32, tag="rd", name="rd")
                        nc.gpsimd.dma_start(rd[:], srec[:])
                        st["rd"] = rd
                return go

            def bcast_block(h, j, st, st1):
                def go():
                    rd = st1["rd"]
                    c = CH * (h % 2)
                    rbc = rbp.tile([64, CH], fp32, tag="rbc", name="rbc")
                    nc.gpsimd.dma_start(
                        rbc[:], rd[0:1, c:c + CH].to_broadcast((64, CH)))
                    st["rbc"] = rbc
                return go

            def mul_block(h, j, st):
                def go():
                    nc.vector.tensor_mul(
                        znT[h // 2][j][64 * (h % 2):64 * (h % 2) + 64, :],
                        st["zps"][0:64, :], st["rbc"][:])
                return go

            def c_mm(m, n, st):
                def go():
                    ps = op.tile([128, CH], fp32, tag="o", name="o")
                    for t in range(2):
                        MM(ps[:],
                           znT[t][m // 4][:, 128 * (m % 4):128 * (m % 4) + 128],
                           woT_s[t][:, CH * n:CH * n + CH],
                           start=(t == 0), stop=(t == 1))
                    st["ps"] = ps
                return go

            def c_evac(m, n, st):
                def go():
                    ot = ob.tile([128, CH], bf16, tag="osb", name="osb")
                    nc.vector.tensor_copy(ot[:], st["ps"][:])
                    nc.sync.dma_start(
                        out_d[128 * m:128 * m + 128, CH * n:CH * n + CH],
                        ot[:])
                return go

            pending = []
            reserve = []

            def pop(n=1):
                for _ in range(n):
                    if pending:
                        pending.pop(0)[1]()

            def queue_group(j, hp, dl):
                heads = (2 * hp, 2 * hp + 1)
                sts = {h: {} for h in heads}
                srec = smp.tile([1, 2 * CH], fp32, tag="srec", name="srec")
                pending.extend([(dl, pv_off(h, j, sts[h])) for h in heads])
                pending.extend([(dl, pv_diag(h, j, sts[h])) for h in heads])
                pending.extend([(dl, evac_block(h, j, sts[h], srec))
                                for h in heads])
                pending.extend([(dl, bcast_block(h, j, sts[h],
                                                 sts[heads[1]]))
                                for h in heads])
                pending.extend([(dl, mul_block(h, j, sts[h])) for h in heads])

            def phase_c_blocks(j, dl):
                cst = [[{} for n in range(2)] for mm_ in range(4)]
                blocks = []
                for mi in range(0, 4, 2):
                    for n in range(2):
                        blocks.append((dl, c_mm(4 * j + mi, n, cst[mi][n])))
                        blocks.append((dl, c_mm(4 * j + mi + 1, n,
                                                cst[mi + 1][n])))
                        blocks.append((dl, c_evac(4 * j + mi, n, cst[mi][n])))
                        blocks.append((dl, c_evac(4 * j + mi + 1, n,
                                                  cst[mi + 1][n])))
                return blocks

            def run_spairs(j, hp):
                heads = (2 * hp, 2 * hp + 1)
                # diagonal pairs first: their exps start the group's
                # serial chain as early as possible
                ips = list(range(2 * j + 2))
                ips = ips[-2:] + ips[:-2]
                for ip in ips:
                    for h in heads:
                        s_pair(h, ip, j)
                    pop(2 if len(pending) >= 8 else 1)

            # j=0 and j=1 groups are tiny (2 and 4 score-pairs) but carry the
            # same ~8us serial chain as big groups; emit all four groups'
            # s_pairs back-to-back so their chains overlap each other.
            for hp in range(2):
                run_spairs(0, hp)
                queue_group(0, hp, 2)
            for hp in range(2):
                run_spairs(1, hp)
                queue_group(1, hp, 3)
            pending += phase_c_blocks(0, 3)
            pending += phase_c_blocks(1, 3)
            for j in (2, 3):
                for hp in range(2):
                    run_spairs(j, hp)
                    while pending:
                        pop()
                    if j == 3 and hp == 1:
                        # ready j=2 filler ahead of the final group's
                        # chain-dependent blocks in the in-order PE queue
                        pending.extend(reserve)
                    queue_group(j, hp, 2 * j + hp + 2)
                    if hp == 1:
                        blocks = phase_c_blocks(j, 2 * j + hp + 2)
                        if j == 2:
                            # hold back half of j=2's phase C as tail filler
                            pending += blocks[:8]
                            reserve = blocks[8:]
                        else:
                            pending += blocks
            while pending:
                pop()

    nc.finalize()
    return nc


_NC_CACHE = {}


def _bias2(b):
    return np.ascontiguousarray(b.reshape(2, 128).T.astype(np.float32))


def kernel(x, attn_mask, wq_w, wq_b, wk_w, wk_b, wv_w, wv_b, wo_w, wo_b):
    import ml_dtypes
    from concourse.bass_utils import run_bass_kernel_spmd

    bf = ml_dtypes.bfloat16
    x = np.asarray(x, np.float32)
    attn_mask = np.asarray(attn_mask)
    wq_w = np.asarray(wq_w, np.float32); wq_b = np.asarray(wq_b, np.float32)
    wk_w = np.asarray(wk_w, np.float32); wk_b = np.asarray(wk_b, np.float32)
    wv_w = np.asarray(wv_w, np.float32); wv_b = np.asarray(wv_b, np.float32)
    wo_w = np.asarray(wo_w, np.float32); wo_b = np.asarray(wo_b, np.float32)

    CT, SST = _rope_tables()
    RT = np.zeros((128, 4, 2, CH), np.float32)
    for j in range(4):
        RT[:, j, 0, :] = CT[:, CH * j:CH * j + CH]
        RT[:, j, 1, :] = SST[:, CH * j:CH * j + CH]
    RT = RT.reshape(128, -1).astype(bf)
    # -1e30 on the strict lower triangle (keys > queries), added to scores
    # in PSUM before exp
    tri01 = np.tril(np.full((128, 128), -1e30, np.float32), -1)

    def dmajor(wt, width):
        # [1024, width] -> [128, 8, width] with d-tiles contiguous per row
        return np.ascontiguousarray(
            wt.reshape(8, 128, width).transpose(1, 0, 2).reshape(128, -1))

    in_maps = []
    for c in range(NC):
        b, g = c // 4, c % 4
        fs = slice(F * g, F * g + F)
        wv = wv_w[fs]
        vb = wv_b[fs]
        wvT = np.zeros((DIM, 272), np.float32)
        vb1 = np.zeros((1, 272), np.float32)
        for h in range(HPC):
            wvT[:, 68 * h:68 * h + 64] = wv[64 * h:64 * h + 64].T
            vb1[0, 68 * h + 64] = 1.0
            vb1[0, 68 * h:68 * h + 64] = vb[64 * h:64 * h + 64]
        # x: [2048, 1024] -> feature-major [1024, 2048] -> [128, j, d, t]
        xb = np.ascontiguousarray(x[b].T).astype(np.float32)
        xb = xb.reshape(8, 128, 4, CH).transpose(1, 2, 0, 3).reshape(128, -1)
        woT = wo_w[:, fs].T  # [256, 1024]
        woT = np.ascontiguousarray(
            woT.reshape(2, 128, DIM).transpose(1, 0, 2).reshape(128, -1))
        in_maps.append({
            "xT": np.ascontiguousarray(xb).astype(bf),
            "wqT": dmajor(np.ascontiguousarray(wq_w[fs].T) / np.float32(8.0),
                          F).astype(bf),
            "wkT": dmajor(np.ascontiguousarray(wk_w[fs].T), F).astype(bf),
            "wvT": dmajor(wvT, 272).astype(bf),
            "woT": woT.astype(bf),
            "qb": _bias2(wq_b[fs] / np.float32(8.0)),
            "kb": _bias2(wk_b[fs]),
            "vb": vb1,
            # col0: 8*mask for fp8 v; col1: mask/4 for bf16 v (so both
            # paths contribute p*v/4 and p/4 to psum; the ratio is exact)
            "maskv": np.ascontiguousarray(np.stack(
                [attn_mask[b].astype(np.float32) * 8.0,
                 attn_mask[b].astype(np.float32) * 0.25],
                1).reshape(16, 128, 2).transpose(1, 0, 2).reshape(128, 32)),
            "RT": RT, "tri": tri01,
        })

    if "nc" not in _NC_CACHE:
        _NC_CACHE["nc"] = _build_program()
    res = run_bass_kernel_spmd(_NC_CACHE["nc"], in_maps, core_ids=list(range(NC)))
    globals()["LAST_RESULTS"] = res

    out = np.zeros((B, S, DIM), np.float32)
    for c in range(NC):
        out[c // 4] += np.asarray(res.results[c]["outp"], np.float32)
    out += wo_b[None, None, :]
    return out


if __name__ == "__main__":
    rng = np.random.default_rng(0)
    ins = {
        "x": rng.standard_normal((B, S, DIM), np.float32),
        "attn_mask": np.ones((B, S), bool),
    }
    for n in ["wq", "wk", "wv", "wo"]:
        ins[n + "_w"] = (rng.standard_normal((DIM, DIM), np.float32) / 32.0)
        ins[n + "_b"] = rng.standard_normal(DIM, np.float32) * 0.01
    o = kernel(**ins)
    print("ran", o.shape, o.dtype)


# revision 62
# speedup vs baseline: 1.0455x; 1.0455x over previous
"""AttentionWithRope Trainium2 Bass kernel (v8).

Sharding: 8 cores = 2 batches x 4 head-groups (4 heads / 256 features each).
Each core computes q/k/v projections for its feature slice on its batch,
RoPE, causal attention for its 4 heads, and a partial output projection.
The host sums the 4 partials per batch and adds wo_b.

v8 over v6 (242us -> ~239us, hardware power-state noise is +/-5%):
  - all DRAM inputs pre-laid-out host-side so every DMA is contiguous per
    partition (>=2KB descriptors, full HBM bandwidth; startup shrinks);
    first x chunk split across two queues.
  - rope: one DVE pass folds bias and casts psum->bf16, ACT does the four
    32-partition swap copies (only ACT does partition-crossing copies
    cheaply), cos/sin muls and final add run as 2x-rate bf16 DVE ops
    (rope tables shipped bf16).
  - causal mask folded in BEFORE exp as a -1e30 triangle add on the scores
    PSUM, so exp emits zeros in the masked region and the pv matmuls have
    no post-exp mask dependency (pv is pure PE filler).
  - qhatT/khatT/znT split into per-chunk tiles so phase B's first scores
    depend only on chunk-0 rope writes, not all of phase A (the tile
    framework tracks dependencies per tile).
  - softmax normalization multiplies straight out of PSUM (no zc copy);
    reciprocals batched per head-pair; 1/sums partition-broadcast via a
    DRAM bounce (gpsimd DMA round trip - measured faster than the Pool
    partition_broadcast custom op).
  - scheduling: j=0/j=1 groups' scores emitted back-to-back as one
    super-group so their ~8us serial chains (exp->pv->evac->recip->
    bounce->mul) overlap; half of j=2's phase C held back as ready PE
    filler ahead of the final group's chain-dependent blocks (the PE
    queue is in-order: a stalled matmul blocks everything behind it).

Rejected by measurement: fp8 DoubleRow projections (3.6e-2 rel err AND
slower), tri/copies on the Pool engine (strided/partition-crossing ops
run ~3x slower there), gpsimd.partition_broadcast (1.2us each), holding
back all of phase C for the tail, descending-j group order.
"""

import numpy as np
from contextlib import ExitStack

DIM, HEADS, HD = 1024, 16, 64
B, S = 2, 2048
NC = 8
HPC = 4          # heads per core
F = HPC * HD     # 256 features per core
CH = 512         # token chunk
ROPE_BASE = 10000.0


def _rope_tables():
    theta = ROPE_BASE ** (-np.arange(0, HD, 2, dtype=np.float32) / HD)  # [32]
    pos = np.arange(S, dtype=np.float32)
    ang = pos[:, None] * theta[None, :]          # [S, 32]
    cos, sin = np.cos(ang).T, np.sin(ang).T      # [32, S]
    CT = np.concatenate([cos, cos, cos, cos], 0).astype(np.float32)      # [128, S]
    SST = np.concatenate([-sin, sin, -sin, sin], 0).astype(np.float32)   # [128, S]
    return CT, SST


def _build_program():
    import concourse.bass as bass
    import concourse.mybir as mybir
    import concourse.tile as tile
    from concourse import bacc

    fp32 = mybir.dt.float32
    bf16 = mybir.dt.bfloat16
    fp8 = mybir.dt.float8e4
    AF = mybir.ActivationFunctionType
    ALU = mybir.AluOpType
    DR = mybir.MatmulPerfMode.DoubleRow
    LN32 = float(np.log(32.0))

    nc = bacc.Bacc("TRN2", target_bir_lowering=False, num_devices=NC)

    from bass_rust import add_dep_helper as _adh
    _prev_mm = [None]

    def MM(*args, **kw):
        bi = nc.tensor.matmul(*args, **kw)
        if _prev_mm[0] is not None:
            _adh(bi.ins, _prev_mm[0].ins, sync=False, reason="pe-order")
        _prev_mm[0] = bi
        return bi

    # ---- DRAM I/O (all pre-laid-out host-side, partition-contiguous) ----
    xT_d = nc.dram_tensor("xT", [128, 4 * 8 * CH], bf16, kind="ExternalInput").ap()
    wqT_d = nc.dram_tensor("wqT", [128, 8 * F], bf16, kind="ExternalInput").ap()
    wkT_d = nc.dram_tensor("wkT", [128, 8 * F], bf16, kind="ExternalInput").ap()
    wvT_d = nc.dram_tensor("wvT", [128, 8 * 272], bf16, kind="ExternalInput").ap()
    woT_d = nc.dram_tensor("woT", [128, 2 * DIM], bf16, kind="ExternalInput").ap()
    qb_d = nc.dram_tensor("qb", [128, 2], fp32, kind="ExternalInput").ap()
    kb_d = nc.dram_tensor("kb", [128, 2], fp32, kind="ExternalInput").ap()
    vb_d = nc.dram_tensor("vb", [1, 272], fp32, kind="ExternalInput").ap()
    mask_d = nc.dram_tensor("maskv", [128, 32], fp32, kind="ExternalInput").ap()
    RT_d = nc.dram_tensor("RT", [128, 4 * 2 * CH], bf16, kind="ExternalInput").ap()
    tri_d = nc.dram_tensor("tri", [128, 128], fp32, kind="ExternalInput").ap()
    out_d = nc.dram_tensor("outp", [S, DIM], bf16, kind="ExternalOutput").ap()

    with tile.TileContext(nc) as tc, ExitStack() as ctx:
        # ---------- persistent SBUF ----------
        const = ctx.enter_context(tc.tile_pool(name="const", bufs=1))
        qk_pool = ctx.enter_context(tc.tile_pool(name="qk", bufs=1))
        v_pool = ctx.enter_context(tc.tile_pool(name="v", bufs=1))
        zn_pool = ctx.enter_context(tc.tile_pool(name="zn", bufs=1))

        tri_s = const.tile([128, 128], fp32, tag="tri", name="tri")
        qb_s = const.tile([128, 2], fp32, tag="qb", name="qb")
        kb_s = const.tile([128, 2], fp32, tag="kb", name="kb")
        vb_s = const.tile([128, 272], fp32, tag="vb", name="vb")
        mask_s = const.tile([128, 32], fp32, tag="maskv", name="maskv")
        nl8_s = const.tile([128, 1], fp32, tag="nl8", name="nl8")
        nc.vector.memset(nl8_s[:], -LN32)
        wo_big = const.tile([128, 2 * DIM], bf16, tag="wobig", name="wobig")
        woT_s = [wo_big[:, DIM * t:DIM * t + DIM] for t in range(2)]

        qhatT = [[qk_pool.tile([128, CH], bf16, tag=f"qhatT{t}_{jc}",
                          name=f"qhatT{t}_{jc}") for jc in range(4)]
                 for t in range(2)]
        khatT = [[qk_pool.tile([128, CH], bf16, tag=f"khatT{t}_{jc}",
                          name=f"khatT{t}_{jc}") for jc in range(4)]
                 for t in range(2)]
        # v layout [p, head, ktile, 65]: per-head k-tile pairs contiguous so
        # the dual-fp8 DoubleRow LDWEIGHTS sees a (k, m) block
        v_big = v_pool.tile([128, 4 * 16 * 80], fp8, tag="vbig", name="vbig")
        v_4d = v_big[:].rearrange("p (h i f) -> p h i f", h=4, i=16)
        nc.vector.memset(v_big[:], 0.0)
        vb_big = v_pool.tile([128, 4 * 16 * 65], bf16, tag="vbbig", name="vbbig")
        vb_4d = vb_big[:].rearrange("p (h i f) -> p h i f", h=4, i=16)
        znT = [[zn_pool.tile([128, CH], bf16, tag=f"znT{t}_{jc}",
                        name=f"znT{t}_{jc}") for jc in range(4)]
               for t in range(2)]

        # ---------- phase A: projections + rope, x streamed by token chunk --
        # producer (matmul) / consumer (evac+rope) stages are emitted one
        # stage apart so the in-order ACT/DVE queues never wait at head on
        # an unfinished PSUM producer.
        with tc.tile_pool(name="wqk", bufs=1) as wp, \
             tc.tile_pool(name="xT", bufs=2) as xp, \
             tc.tile_pool(name="ppsum", bufs=3, space="PSUM") as pp, \
             tc.tile_pool(name="ropetmp", bufs=5) as rp:

            wq_big = wp.tile([128, 8 * F], bf16, tag="wqbig", name="wqbig")
            wk_big = wp.tile([128, 8 * F], bf16, tag="wkbig", name="wkbig")
            wv_big = wp.tile([128, 8 * 272], bf16, tag="wvbig", name="wvbig")
            wq_s = [wq_big[:, F * d:F * d + F] for d in range(8)]
            wk_s = [wk_big[:, F * d:F * d + F] for d in range(8)]
            wv_s = [wv_big[:, 272 * d:272 * d + 272] for d in range(8)]

            stages = []

            def qk_stage(x_s, w_s, b_s, dst, fc, ctb, sstb, cs):
                ps = [None]

                def produce():
                    ps[0] = pp.tile([128, CH], fp32, tag="proj", name="proj")
                    for d in range(8):
                        MM(ps[0][:], w_s[d][:, 128 * fc:128 * fc + 128],
                           x_s[d][:], start=(d == 0), stop=(d == 7))

                def consume():
                    # pc = bf16(ps + bias); swaps on ACT; muls/add 2x bf16 DVE
                    pc = rp.tile([128, CH], bf16, tag="pc", name="pc")
                    nc.vector.tensor_scalar_add(pc[:], ps[0][:],
                                                b_s[:, fc:fc + 1])
                    sw = rp.tile([128, CH], bf16, tag="sw", name="sw")
                    # partition-swap copies: ACT only — DVE/Pool take ~1.9us
                    # for partition-crossing copies (slow shuffle path)
                    for o, so in ((0, 32), (32, 0), (64, 96), (96, 64)):
                        nc.scalar.activation(sw[o:o + 32, :],
                                             pc[so:so + 32, :], AF.Identity)
                    qct = rp.tile([128, CH], bf16, tag="qct", name="qct")
                    nc.vector.tensor_mul(qct[:], pc[:], ctb)
                    nc.vector.tensor_mul(sw[:], sw[:], sstb)
                    nc.vector.tensor_add(dst[fc][:], qct[:], sw[:])
                return produce, consume

            def v_stage(x_s, t):
                ps = [None]

                def produce():
                    ps[0] = pp.tile([128, 272], fp32, tag="vproj",
                                    name="vproj")
                    for d in range(8):
                        MM(ps[0][:], x_s[d][:, 128 * (t % 4):128 * (t % 4) + 128],
                           wv_s[d][:], start=(d == 0), stop=(d == 7))

                def consume():
                    vt = rp.tile([128, 272], fp32, tag="vt", name="vt")
                    nc.vector.tensor_add(vt[:], ps[0][:], vb_s[:])
                    vt3 = vt[:].rearrange("p (h f) -> p h f", h=4)
                    nc.vector.tensor_scalar_mul(v_4d[:, :, t, 0:68], vt3,
                                                mask_s[:, 2 * t:2 * t + 1])
                    nc.vector.tensor_scalar_mul(vb_4d[:, :, t, :],
                                                vt3[:, :, 0:65],
                                                mask_s[:, 2 * t + 1:2 * t + 2])
                return produce, consume

            for j in range(4):
                cs = slice(CH * j, CH * j + CH)
                xc = xp.tile([128, 8 * CH], bf16, tag="xc", name="xc")
                x_s = [xc[:, CH * d:CH * d + CH] for d in range(8)]
                if j == 0:
                    # split the first chunk across two queues: halves the
                    # DMA latency gating the very first matmul
                    nc.scalar.dma_start(xc[:, 0:4 * CH],
                                        xT_d[:, 0:4 * CH])
                    nc.gpsimd.dma_start(xc[:, 4 * CH:8 * CH],
                                        xT_d[:, 4 * CH:8 * CH])
                else:
                    nc.sync.dma_start(
                        xc[:], xT_d[:, 8 * CH * j:8 * CH * j + 8 * CH])
                rt = xp.tile([128, 2 * CH], bf16, tag="rt", name="rt")
                ctb, sstb = rt[:, 0:CH], rt[:, CH:2 * CH]
                (nc.gpsimd if j == 0 else nc.sync).dma_start(
                    rt[:], RT_d[:, 2 * CH * j:2 * CH * j + 2 * CH])
                if j == 0:
                    nc.gpsimd.dma_start(qb_s[:], qb_d[:])
                    nc.gpsimd.dma_start(kb_s[:], kb_d[:])
                    nc.sync.dma_start(wq_big[:], wqT_d[:])
                    nc.sync.dma_start(wk_big[:], wkT_d[:])
                    nc.scalar.dma_start(wv_big[:], wvT_d[:])
                    nc.gpsimd.dma_start(vb_s[:],
                                        vb_d[0:1, :].to_broadcast((128, 272)))
                    nc.gpsimd.dma_start(mask_s[:], mask_d[:])
                    nc.gpsimd.dma_start(tri_s[:], tri_d[:])
                if j == 1:
                    nc.sync.dma_start(wo_big[:], woT_d[:])

                for w_s, b_s, dstj in ((wq_s, qb_s, qhatT),
                                       (wk_s, kb_s, khatT)):
                    for fc in range(2):
                        stages.append(qk_stage(x_s, w_s, b_s,
                                               [dstj[0][j], dstj[1][j]], fc,
                                               ctb, sstb, cs))
                for tt in range(4):
                    stages.append(v_stage(x_s, 4 * j + tt))
                # emit this chunk's stages one step out of phase
                while len(stages) > 1:
                    prod, cons = stages.pop(0)
                    prod()
                    stages[0][0]()
                    # swap so the pending producer isn't re-run
                    stages[0] = (lambda: None, stages[0][1])
                    cons()
            prod, cons = stages.pop(0)
            prod()
            cons()

        # ---------- phase B + C: software-pipelined attention ----------
        # every PSUM consumer runs as its own queued block, popped a few
        # score-slots after its producer, so no engine queue blocks at head.
        with tc.tile_pool(name="spsum", bufs=4, space="PSUM") as sp, \
             tc.tile_pool(name="zpsum", bufs=2, space="PSUM") as zp, \
             tc.tile_pool(name="opsum", bufs=2, space="PSUM") as op, \
             tc.tile_pool(name="pT", bufs=1) as ptp, \
             tc.tile_pool(name="sums", bufs=6) as smp, \
             tc.tile_pool(name="rbc", bufs=6) as rbp, \
             tc.tile_pool(name="rdram", bufs=6, space="DRAM") as rdp, \
             tc.tile_pool(name="osbuf", bufs=4) as ob:

            pts = {}

            def s_pair(h, ip, j):
                tH, rH = h // 2, 64 * (h % 2)
                diag = 2 * ip + 1 - 4 * j >= 1
                pb = ptp.tile([128, 1024], bf16 if diag else fp8,
                              tag=f"p{h}_{ip}_{int(diag)}", name="p")
                pts[(h, ip, j)] = pb
                if not diag:
                    pbi = pb[:].rearrange("p (n k) -> p k n", k=2)
                # single-bank score tiles (bufs=4): the PE's psum-reuse wait
                # reaches 4 tiles back (~3 exps of slack) instead of 2, so
                # the matmul stream no longer stalls on each pair's exp;
                # per-tile exps also trim the diagonal overhang exactly
                for ii in range(2):
                    i = 2 * ip + ii
                    d = i - 4 * j
                    c0 = max(0, 128 * d)
                    sps = sp.tile([128, 512], fp32, tag="s", name="s")
                    MM(sps[:, c0:512],
                       khatT[tH][i // 4][rH:rH + 64,
                                         128 * (i % 4):128 * (i % 4) + 128],
                       qhatT[tH][j][rH:rH + 64, c0:CH],
                       start=True, stop=True)
                    if diag and d >= 0:
                        # causal mask folded in BEFORE exp: -1e30 on the
                        # strict lower triangle of the diagonal 128-block
                        nc.vector.tensor_add(sps[:, c0:c0 + 128],
                                             sps[:, c0:c0 + 128],
                                             tri_s[:])
                    # off-diag: p/32 in fp8 (overflow-safe; the 8x in fp8 v
                    # makes the off-diag contribution p*v/4, matched by the
                    # 1/4-scaled bf16 diag path; normalization cancels it);
                    # diag: plain exp in bf16 (row sums can't underflow)
                    if diag:
                        nc.scalar.activation(
                            pb[:, 512 * ii + c0:512 * ii + 512],
                            sps[:, c0:512], AF.Exp)
                    else:
                        nc.scalar.activation(pbi[:, ii, :], sps[:, 0:512],
                                             AF.Exp, bias=nl8_s[:, 0:1])

            def pv_off(h, j, st):
                # off-diagonal DR matmuls: no tri-mask dependency, their exp
                # finished long ago -> pure PE filler
                def go():
                    if j == 0:
                        return
                    zps = zp.tile([80, CH], fp32, tag="z", name="z")
                    st["zps"] = zps
                    for ip in range(2 * j):
                        pbv = pts[(h, ip, j)][:].rearrange(
                            "p (n k) -> p k n", k=2)
                        MM(zps[:, 0:CH],
                           v_4d[:, h, 2 * ip:2 * ip + 2, :],
                           pbv[:, :, 0:CH],
                           start=(ip == 0), stop=False,
                           perf_mode=DR)
                return go

            def pv_diag(h, j, st):
                def go():
                    if j == 0:
                        zps = zp.tile([80, CH], fp32, tag="z", name="z")
                        st["zps"] = zps
                    zps = st["zps"]
                    first = (j == 0)
                    for k, ip in enumerate((2 * j, 2 * j + 1)):
                        for ii in range(2):
                            i = 2 * ip + ii
                            c0 = 128 * (i - 4 * j)
                            MM(zps[0:65, c0:CH],
                               vb_4d[:, h, i, :],
                               pts[(h, ip, j)][:, 512 * ii + c0:512 * ii + 512],
                               start=(first and k == 0 and ii == 0),
                               stop=(k == 1 and ii == 1))
                return go

            def evac_block(h, j, st, srec):
                def go():
                    c = CH * (h % 2)
                    nc.vector.tensor_copy(srec[0:1, c:c + CH],
                                          st["zps"][64:65, :])
                    if c:
                        # both heads' sums in one custom-DVE recip call
                        nc.vector.reciprocal_approx_fast(srec[:], srec[:])
                        rd = rdp.tile([1, 2 * CH], fp32, tag="rd", name="rd")
                        nc.gpsimd.dma_start(rd[:], srec[:])
                        st["rd"] = rd
                return go

            def bcast_block(h, j, st, st1):
                def go():
                    rd = st1["rd"]
                    c = CH * (h % 2)
                    rbc = rbp.tile([64, CH], fp32, tag="rbc", name="rbc")
                    nc.gpsimd.dma_start(
                        rbc[:], rd[0:1, c:c + CH].to_broadcast((64, CH)))
                    st["rbc"] = rbc
                return go

            def mul_block(h, j, st):
                def go():
                    nc.vector.tensor_mul(
                        znT[h // 2][j][64 * (h % 2):64 * (h % 2) + 64, :],
                        st["zps"][0:64, :], st["rbc"][:])
                return go

            def c_mm(m, n, st):
                def go():
                    ps = op.tile([128, CH], fp32, tag="o", name="o")
                    for t in range(2):
                        MM(ps[:],
                           znT[t][m // 4][:, 128 * (m % 4):128 * (m % 4) + 128],
                           woT_s[t][:, CH * n:CH * n + CH],
                           start=(t == 0), stop=(t == 1))
                    st["ps"] = ps
                return go

            def c_evac(m, n, st):
                def go():
                    ot = ob.tile([128, CH], bf16, tag="osb", name="osb")
                    nc.vector.tensor_copy(ot[:], st["ps"][:])
                    nc.sync.dma_start(
                        out_d[128 * m:128 * m + 128, CH * n:CH * n + CH],
                        ot[:])
                return go

            pending = []
            reserve = []

            def pop(n=1):
                for _ in range(n):
                    if pending:
                        pending.pop(0)[1]()

            def queue_group(j, hp, dl):
                heads = (2 * hp, 2 * hp + 1)
                sts = {h: {} for h in heads}
                srec = smp.tile([1, 2 * CH], fp32, tag="srec", name="srec")
                pending.extend([(dl, pv_off(h, j, sts[h])) for h in heads])
                pending.extend([(dl, pv_diag(h, j, sts[h])) for h in heads])
                pending.extend([(dl, evac_block(h, j, sts[h], srec))
                                for h in heads])
                pending.extend([(dl, bcast_block(h, j, sts[h],
                                                 sts[heads[1]]))
                                for h in heads])
                pending.extend([(dl, mul_block(h, j, sts[h])) for h in heads])

            def phase_c_blocks(j, dl):
                cst = [[{} for n in range(2)] for mm_ in range(4)]
                blocks = []
                for mi in range(0, 4, 2):
                    for n in range(2):
                        blocks.append((dl, c_mm(4 * j + mi, n, cst[mi][n])))
                        blocks.append((dl, c_mm(4 * j + mi + 1, n,
                                                cst[mi + 1][n])))
                        blocks.append((dl, c_evac(4 * j + mi, n, cst[mi][n])))
                        blocks.append((dl, c_evac(4 * j + mi + 1, n,
                                                  cst[mi + 1][n])))
                return blocks

            def run_spairs(j, hp):
                heads = (2 * hp, 2 * hp + 1)
                # diagonal pairs first: their exps start the group's
                # serial chain as early as possible
                ips = list(range(2 * j + 2))
                ips = ips[-2:] + ips[:-2]
                for ip in ips:
                    for h in heads:
                        s_pair(h, ip, j)
                    pop(2 if len(pending) >= 8 else 1)

            # j=0 and j=1 groups are tiny (2 and 4 score-pairs) but carry the
            # same ~8us serial chain as big groups; emit all four groups'
            # s_pairs back-to-back so their chains overlap each other.
            for hp in range(2):
                run_spairs(0, hp)
                queue_group(0, hp, 2)
            for hp in range(2):
                run_spairs(1, hp)
                queue_group(1, hp, 3)
            pending += phase_c_blocks(0, 3)
            pending += phase_c_blocks(1, 3)
            for j in (2, 3):
                for hp in range(2):
                    run_spairs(j, hp)
                    while pending:
                        pop()
                    if j == 3 and hp == 1:
                        # ready j=2 filler ahead of the final group's
                        # chain-dependent blocks in the in-order PE queue
                        pending.extend(reserve)
                    queue_group(j, hp, 2 * j + hp + 2)
                    if hp == 1:
                        blocks = phase_c_blocks(j, 2 * j + hp + 2)
                        if j == 2:
                            # hold back half of j=2's phase C as tail filler
                            pending += blocks[:8]
                            reserve = blocks[8:]
                        else:
                            pending += blocks
            while pending:
                pop()

    nc.finalize()
    return nc


_NC_CACHE = {}


def _bias2(b):
    return np.ascontiguousarray(b.reshape(2, 128).T.astype(np.float32))


def kernel(x, attn_mask, wq_w, wq_b, wk_w, wk_b, wv_w, wv_b, wo_w, wo_b):
    import ml_dtypes
    from concourse.bass_utils import run_bass_kernel_spmd

    bf = ml_dtypes.bfloat16
    x = np.asarray(x, np.float32)
    attn_mask = np.asarray(attn_mask)
    wq_w = np.asarray(wq_w, np.float32); wq_b = np.asarray(wq_b, np.float32)
    wk_w = np.asarray(wk_w, np.float32); wk_b = np.asarray(wk_b, np.float32)
    wv_w = np.asarray(wv_w, np.float32); wv_b = np.asarray(wv_b, np.float32)
    wo_w = np.asarray(wo_w, np.float32); wo_b = np.asarray(wo_b, np.float32)

    CT, SST = _rope_tables()
    RT = np.zeros((128, 4, 2, CH), np.float32)
    for j in range(4):
        RT[:, j, 0, :] = CT[:, CH * j:CH * j + CH]
        RT[:, j, 1, :] = SST[:, CH * j:CH * j + CH]
    RT = RT.reshape(128, -1).astype(bf)
    # -1e30 on the strict lower triangle (keys > queries), added to scores
    # in PSUM before exp
    tri01 = np.tril(np.full((128, 128), -1e30, np.float32), -1)

    def dmajor(wt, width):
        # [1024, width] -> [128, 8, width] with d-tiles contiguous per row
        return np.ascontiguousarray(
            wt.reshape(8, 128, width).transpose(1, 0, 2).reshape(128, -1))

    in_maps = []
    for c in range(NC):
        b, g = c // 4, c % 4
        fs = slice(F * g, F * g + F)
        wv = wv_w[fs]
        vb = wv_b[fs]
        wvT = np.zeros((DIM, 272), np.float32)
        vb1 = np.zeros((1, 272), np.float32)
        for h in range(HPC):
            wvT[:, 68 * h:68 * h + 64] = wv[64 * h:64 * h + 64].T
            vb1[0, 68 * h + 64] = 1.0
            vb1[0, 68 * h:68 * h + 64] = vb[64 * h:64 * h + 64]
        # x: [2048, 1024] -> feature-major [1024, 2048] -> [128, j, d, t]
        xb = np.ascontiguousarray(x[b].T).astype(np.float32)
        xb = xb.reshape(8, 128, 4, CH).transpose(1, 2, 0, 3).reshape(128, -1)
        woT = wo_w[:, fs].T  # [256, 1024]
        woT = np.ascontiguousarray(
            woT.reshape(2, 128, DIM).transpose(1, 0, 2).reshape(128, -1))
        in_maps.append({
            "xT": np.ascontiguousarray(xb).astype(bf),
            "wqT": dmajor(np.ascontiguousarray(wq_w[fs].T) / np.float32(8.0),
                          F).astype(bf),
            "wkT": dmajor(np.ascontiguousarray(wk_w[fs].T), F).astype(bf),
            "wvT": dmajor(wvT, 272).astype(bf),
            "woT": woT.astype(bf),
            "qb": _bias2(wq_b[fs] / np.float32(8.0)),
            "kb": _bias2(wk_b[fs]),
            "vb": vb1,
            # col0: 8*mask for fp8 v; col1: mask/4 for bf16 v (so both
            # paths contribute p*v/4 and p/4 to psum; the ratio is exact)
            "maskv": np.ascontiguousarray(np.stack(
                [attn_mask[b].astype(np.float32) * 8.0,
                 attn_mask[b].astype(np.float32) * 0.25],
                1).reshape(16, 128, 2).transpose(1, 0, 2).reshape(128, 32)),
            "RT": RT, "tri": tri01,
        })

    if "nc" not in _NC_CACHE:
        _NC_CACHE["nc"] = _build_program()
    res = run_bass_kernel_spmd(_NC_CACHE["nc"], in_maps, core_ids=list(range(NC)))
    globals()["LAST_RESULTS"] = res

    out = np.zeros((B, S, DIM), np.float32)
    for c in range(NC):
        out[c // 4] += np.asarray(res.results[c]["outp"], np.float32)
    out += wo_b[None, None, :]
    return out


if __name__ == "__main__":
    rng = np.random.default_rng(0)
    ins = {
        "x": rng.standard_normal((B, S, DIM), np.float32),
        "attn_mask": np.ones((B, S), bool),
    }
    for n in ["wq", "wk", "wv", "wo"]:
        ins[n + "_w"] = (rng.standard_normal((DIM, DIM), np.float32) / 32.0)
        ins[n + "_b"] = rng.standard_normal(DIM, np.float32) * 0.01
    o = kernel(**ins)
    print("ran", o.shape, o.dtype)


# revision 63
# speedup vs baseline: 1.0585x; 1.0124x over previous
"""AttentionWithRope Trainium2 Bass kernel (v8).

Sharding: 8 cores = 2 batches x 4 head-groups (4 heads / 256 features each).
Each core computes q/k/v projections for its feature slice on its batch,
RoPE, causal attention for its 4 heads, and a partial output projection.
The host sums the 4 partials per batch and adds wo_b.

v8 over v6 (242us -> ~239us, hardware power-state noise is +/-5%):
  - all DRAM inputs pre-laid-out host-side so every DMA is contiguous per
    partition (>=2KB descriptors, full HBM bandwidth; startup shrinks);
    first x chunk split across two queues.
  - rope: one DVE pass folds bias and casts psum->bf16, ACT does the four
    32-partition swap copies (only ACT does partition-crossing copies
    cheaply), cos/sin muls and final add run as 2x-rate bf16 DVE ops
    (rope tables shipped bf16).
  - causal mask folded in BEFORE exp as a -1e30 triangle add on the scores
    PSUM, so exp emits zeros in the masked region and the pv matmuls have
    no post-exp mask dependency (pv is pure PE filler).
  - qhatT/khatT/znT split into per-chunk tiles so phase B's first scores
    depend only on chunk-0 rope writes, not all of phase A (the tile
    framework tracks dependencies per tile).
  - softmax normalization multiplies straight out of PSUM (no zc copy);
    reciprocals batched per head-pair; 1/sums partition-broadcast via a
    DRAM bounce (gpsimd DMA round trip - measured faster than the Pool
    partition_broadcast custom op).
  - scheduling: j=0/j=1 groups' scores emitted back-to-back as one
    super-group so their ~8us serial chains (exp->pv->evac->recip->
    bounce->mul) overlap; half of j=2's phase C held back as ready PE
    filler ahead of the final group's chain-dependent blocks (the PE
    queue is in-order: a stalled matmul blocks everything behind it).

Rejected by measurement: fp8 DoubleRow projections (3.6e-2 rel err AND
slower), tri/copies on the Pool engine (strided/partition-crossing ops
run ~3x slower there), gpsimd.partition_broadcast (1.2us each), holding
back all of phase C for the tail, descending-j group order.
"""

import numpy as np
from contextlib import ExitStack

DIM, HEADS, HD = 1024, 16, 64
B, S = 2, 2048
NC = 8
HPC = 4          # heads per core
F = HPC * HD     # 256 features per core
CH = 512         # token chunk
ROPE_BASE = 10000.0


def _rope_tables():
    theta = ROPE_BASE ** (-np.arange(0, HD, 2, dtype=np.float32) / HD)  # [32]
    pos = np.arange(S, dtype=np.float32)
    ang = pos[:, None] * theta[None, :]          # [S, 32]
    cos, sin = np.cos(ang).T, np.sin(ang).T      # [32, S]
    CT = np.concatenate([cos, cos, cos, cos], 0).astype(np.float32)      # [128, S]
    SST = np.concatenate([-sin, sin, -sin, sin], 0).astype(np.float32)   # [128, S]
    return CT, SST


def _build_program():
    import concourse.bass as bass
    import concourse.mybir as mybir
    import concourse.tile as tile
    from concourse import bacc

    fp32 = mybir.dt.float32
    bf16 = mybir.dt.bfloat16
    fp8 = mybir.dt.float8e4
    AF = mybir.ActivationFunctionType
    ALU = mybir.AluOpType
    DR = mybir.MatmulPerfMode.DoubleRow
    LN32 = float(np.log(32.0))

    nc = bacc.Bacc("TRN2", target_bir_lowering=False, num_devices=NC)

    from bass_rust import add_dep_helper as _adh
    _prev_mm = [None]

    def MM(*args, **kw):
        bi = nc.tensor.matmul(*args, **kw)
        if _prev_mm[0] is not None:
            _adh(bi.ins, _prev_mm[0].ins, sync=False, reason="pe-order")
        _prev_mm[0] = bi
        return bi

    # ---- DRAM I/O (all pre-laid-out host-side, partition-contiguous) ----
    xT_d = nc.dram_tensor("xT", [128, 4 * 8 * CH], bf16, kind="ExternalInput").ap()
    wqT_d = nc.dram_tensor("wqT", [128, 8 * F], bf16, kind="ExternalInput").ap()
    wkT_d = nc.dram_tensor("wkT", [128, 8 * F], bf16, kind="ExternalInput").ap()
    wvT_d = nc.dram_tensor("wvT", [128, 8 * 272], bf16, kind="ExternalInput").ap()
    woT_d = nc.dram_tensor("woT", [128, 2 * DIM], bf16, kind="ExternalInput").ap()
    qb_d = nc.dram_tensor("qb", [128, 2], fp32, kind="ExternalInput").ap()
    kb_d = nc.dram_tensor("kb", [128, 2], fp32, kind="ExternalInput").ap()
    vb_d = nc.dram_tensor("vb", [1, 272], fp32, kind="ExternalInput").ap()
    mask_d = nc.dram_tensor("maskv", [128, 32], fp32, kind="ExternalInput").ap()
    RT_d = nc.dram_tensor("RT", [128, 4 * 2 * CH], bf16, kind="ExternalInput").ap()
    tri_d = nc.dram_tensor("tri", [128, 128], fp32, kind="ExternalInput").ap()
    out_d = nc.dram_tensor("outp", [S, DIM], bf16, kind="ExternalOutput").ap()

    with tile.TileContext(nc) as tc, ExitStack() as ctx:
        # ---------- persistent SBUF ----------
        const = ctx.enter_context(tc.tile_pool(name="const", bufs=1))
        qk_pool = ctx.enter_context(tc.tile_pool(name="qk", bufs=1))
        v_pool = ctx.enter_context(tc.tile_pool(name="v", bufs=1))
        zn_pool = ctx.enter_context(tc.tile_pool(name="zn", bufs=1))

        tri_s = const.tile([128, 128], fp32, tag="tri", name="tri")
        qb_s = const.tile([128, 2], fp32, tag="qb", name="qb")
        kb_s = const.tile([128, 2], fp32, tag="kb", name="kb")
        vb_s = const.tile([128, 272], fp32, tag="vb", name="vb")
        mask_s = const.tile([128, 32], fp32, tag="maskv", name="maskv")
        nl8_s = const.tile([128, 1], fp32, tag="nl8", name="nl8")
        nc.vector.memset(nl8_s[:], -LN32)
        wo_big = const.tile([128, 2 * DIM], bf16, tag="wobig", name="wobig")
        woT_s = [wo_big[:, DIM * t:DIM * t + DIM] for t in range(2)]

        qhatT = [[qk_pool.tile([128, CH], bf16, tag=f"qhatT{t}_{jc}",
                          name=f"qhatT{t}_{jc}") for jc in range(4)]
                 for t in range(2)]
        khatT = [[qk_pool.tile([128, CH], bf16, tag=f"khatT{t}_{jc}",
                          name=f"khatT{t}_{jc}") for jc in range(4)]
                 for t in range(2)]
        # v layout [p, head, ktile, 65]: per-head k-tile pairs contiguous so
        # the dual-fp8 DoubleRow LDWEIGHTS sees a (k, m) block
        v_big = v_pool.tile([128, 4 * 16 * 80], fp8, tag="vbig", name="vbig")
        v_4d = v_big[:].rearrange("p (h i f) -> p h i f", h=4, i=16)
        nc.vector.memset(v_big[:], 0.0)
        vb_big = v_pool.tile([128, 4 * 16 * 65], bf16, tag="vbbig", name="vbbig")
        vb_4d = vb_big[:].rearrange("p (h i f) -> p h i f", h=4, i=16)
        znT = [[zn_pool.tile([128, CH], bf16, tag=f"znT{t}_{jc}",
                        name=f"znT{t}_{jc}") for jc in range(4)]
               for t in range(2)]

        # ---------- phase A: projections + rope, x streamed by token chunk --
        # producer (matmul) / consumer (evac+rope) stages are emitted one
        # stage apart so the in-order ACT/DVE queues never wait at head on
        # an unfinished PSUM producer.
        with tc.tile_pool(name="wqk", bufs=1) as wp, \
             tc.tile_pool(name="xT", bufs=2) as xp, \
             tc.tile_pool(name="ppsum", bufs=3, space="PSUM") as pp, \
             tc.tile_pool(name="ropetmp", bufs=5) as rp:

            wq_big = wp.tile([128, 8 * F], bf16, tag="wqbig", name="wqbig")
            wk_big = wp.tile([128, 8 * F], bf16, tag="wkbig", name="wkbig")
            wv_big = wp.tile([128, 8 * 272], bf16, tag="wvbig", name="wvbig")
            wq_s = [wq_big[:, F * d:F * d + F] for d in range(8)]
            wk_s = [wk_big[:, F * d:F * d + F] for d in range(8)]
            wv_s = [wv_big[:, 272 * d:272 * d + 272] for d in range(8)]

            stages = []

            def qk_stage(x_s, w_s, b_s, dst, fc, ctb, sstb, cs):
                ps = [None]

                def produce():
                    ps[0] = pp.tile([128, CH], fp32, tag="proj", name="proj")
                    for d in range(8):
                        MM(ps[0][:], w_s[d][:, 128 * fc:128 * fc + 128],
                           x_s[d][:], start=(d == 0), stop=(d == 7))

                def consume():
                    # pc = bf16(ps + bias); swaps on ACT; muls/add 2x bf16 DVE
                    pc = rp.tile([128, CH], bf16, tag="pc", name="pc")
                    nc.vector.tensor_scalar_add(pc[:], ps[0][:],
                                                b_s[:, fc:fc + 1])
                    sw = rp.tile([128, CH], bf16, tag="sw", name="sw")
                    # partition-swap copies: ACT only — DVE/Pool take ~1.9us
                    # for partition-crossing copies (slow shuffle path)
                    for o, so in ((0, 32), (32, 0), (64, 96), (96, 64)):
                        nc.scalar.activation(sw[o:o + 32, :],
                                             pc[so:so + 32, :], AF.Identity)
                    qct = rp.tile([128, CH], bf16, tag="qct", name="qct")
                    nc.vector.tensor_mul(qct[:], pc[:], ctb)
                    nc.vector.tensor_mul(sw[:], sw[:], sstb)
                    nc.vector.tensor_add(dst[fc][:], qct[:], sw[:])
                return produce, consume

            def v_stage(x_s, t):
                ps = [None]

                def produce():
                    ps[0] = pp.tile([128, 272], fp32, tag="vproj",
                                    name="vproj")
                    for d in range(8):
                        MM(ps[0][:], x_s[d][:, 128 * (t % 4):128 * (t % 4) + 128],
                           wv_s[d][:], start=(d == 0), stop=(d == 7))

                def consume():
                    vt = rp.tile([128, 272], fp32, tag="vt", name="vt")
                    nc.vector.tensor_add(vt[:], ps[0][:], vb_s[:])
                    vt3 = vt[:].rearrange("p (h f) -> p h f", h=4)
                    nc.vector.tensor_scalar_mul(v_4d[:, :, t, 0:68], vt3,
                                                mask_s[:, 2 * t:2 * t + 1])
                    nc.vector.tensor_scalar_mul(vb_4d[:, :, t, :],
                                                vt3[:, :, 0:65],
                                                mask_s[:, 2 * t + 1:2 * t + 2])
                return produce, consume

            for j in range(4):
                cs = slice(CH * j, CH * j + CH)
                xc = xp.tile([128, 8 * CH], bf16, tag="xc", name="xc")
                x_s = [xc[:, CH * d:CH * d + CH] for d in range(8)]
                if j == 0:
                    # split the first chunk across two queues: halves the
                    # DMA latency gating the very first matmul
                    nc.scalar.dma_start(xc[:, 0:4 * CH],
                                        xT_d[:, 0:4 * CH])
                    nc.gpsimd.dma_start(xc[:, 4 * CH:8 * CH],
                                        xT_d[:, 4 * CH:8 * CH])
                else:
                    nc.sync.dma_start(
                        xc[:], xT_d[:, 8 * CH * j:8 * CH * j + 8 * CH])
                rt = xp.tile([128, 2 * CH], bf16, tag="rt", name="rt")
                ctb, sstb = rt[:, 0:CH], rt[:, CH:2 * CH]
                (nc.gpsimd if j == 0 else nc.sync).dma_start(
                    rt[:], RT_d[:, 2 * CH * j:2 * CH * j + 2 * CH])
                if j == 0:
                    nc.gpsimd.dma_start(qb_s[:], qb_d[:])
                    nc.gpsimd.dma_start(kb_s[:], kb_d[:])
                    nc.sync.dma_start(wq_big[:], wqT_d[:])
                    nc.sync.dma_start(wk_big[:], wkT_d[:])
                    nc.scalar.dma_start(wv_big[:], wvT_d[:])
                    nc.gpsimd.dma_start(vb_s[:],
                                        vb_d[0:1, :].to_broadcast((128, 272)))
                    nc.gpsimd.dma_start(mask_s[:], mask_d[:])
                    nc.gpsimd.dma_start(tri_s[:], tri_d[:])
                if j == 1:
                    nc.sync.dma_start(wo_big[:], woT_d[:])

                for w_s, b_s, dstj in ((wq_s, qb_s, qhatT),
                                       (wk_s, kb_s, khatT)):
                    for fc in range(2):
                        stages.append(qk_stage(x_s, w_s, b_s,
                                               [dstj[0][j], dstj[1][j]], fc,
                                               ctb, sstb, cs))
                for tt in range(4):
                    stages.append(v_stage(x_s, 4 * j + tt))
                # emit this chunk's stages one step out of phase
                while len(stages) > 1:
                    prod, cons = stages.pop(0)
                    prod()
                    stages[0][0]()
                    # swap so the pending producer isn't re-run
                    stages[0] = (lambda: None, stages[0][1])
                    cons()
            prod, cons = stages.pop(0)
            prod()
            cons()

        # ---------- phase B + C: software-pipelined attention ----------
        # every PSUM consumer runs as its own queued block, popped a few
        # score-slots after its producer, so no engine queue blocks at head.
        with tc.tile_pool(name="spsum", bufs=4, space="PSUM") as sp, \
             tc.tile_pool(name="zpsum", bufs=2, space="PSUM") as zp, \
             tc.tile_pool(name="opsum", bufs=2, space="PSUM") as op, \
             tc.tile_pool(name="pT", bufs=1) as ptp, \
             tc.tile_pool(name="sums", bufs=4) as smp, \
             tc.tile_pool(name="rbc", bufs=4) as rbp, \
             tc.tile_pool(name="rdram", bufs=4, space="DRAM") as rdp, \
             tc.tile_pool(name="osbuf", bufs=4) as ob:

            pts = {}

            def s_pair(h, ip, j):
                tH, rH = h // 2, 64 * (h % 2)
                diag = 2 * ip + 1 - 4 * j >= 1
                pb = ptp.tile([128, 1024], bf16 if diag else fp8,
                              tag=f"p{h}_{ip}_{int(diag)}", name="p")
                pts[(h, ip, j)] = pb
                if not diag:
                    pbi = pb[:].rearrange("p (n k) -> p k n", k=2)
                # single-bank score tiles (bufs=4): the PE's psum-reuse wait
                # reaches 4 tiles back (~3 exps of slack) instead of 2, so
                # the matmul stream no longer stalls on each pair's exp;
                # per-tile exps also trim the diagonal overhang exactly
                for ii in range(2):
                    i = 2 * ip + ii
                    d = i - 4 * j
                    c0 = max(0, 128 * d)
                    sps = sp.tile([128, 512], fp32, tag="s", name="s")
                    MM(sps[:, c0:512],
                       khatT[tH][i // 4][rH:rH + 64,
                                         128 * (i % 4):128 * (i % 4) + 128],
                       qhatT[tH][j][rH:rH + 64, c0:CH],
                       start=True, stop=True)
                    if diag and d >= 0:
                        # causal mask folded in BEFORE exp: -1e30 on the
                        # strict lower triangle of the diagonal 128-block
                        nc.vector.tensor_add(sps[:, c0:c0 + 128],
                                             sps[:, c0:c0 + 128],
                                             tri_s[:])
                    # off-diag: p/32 in fp8 (overflow-safe; the 8x in fp8 v
                    # makes the off-diag contribution p*v/4, matched by the
                    # 1/4-scaled bf16 diag path; normalization cancels it);
                    # diag: plain exp in bf16 (row sums can't underflow)
                    if diag:
                        nc.scalar.activation(
                            pb[:, 512 * ii + c0:512 * ii + 512],
                            sps[:, c0:512], AF.Exp)
                    else:
                        nc.scalar.activation(pbi[:, ii, :], sps[:, 0:512],
                                             AF.Exp, bias=nl8_s[:, 0:1])

            def pv_off(h, j, st):
                # off-diagonal DR matmuls: no tri-mask dependency, their exp
                # finished long ago -> pure PE filler
                def go():
                    if j == 0:
                        return
                    zps = zp.tile([80, CH], fp32, tag="z", name="z")
                    st["zps"] = zps
                    for ip in range(2 * j):
                        pbv = pts[(h, ip, j)][:].rearrange(
                            "p (n k) -> p k n", k=2)
                        MM(zps[:, 0:CH],
                           v_4d[:, h, 2 * ip:2 * ip + 2, :],
                           pbv[:, :, 0:CH],
                           start=(ip == 0), stop=False,
                           perf_mode=DR)
                return go

            def pv_diag(h, j, st):
                def go():
                    if j == 0:
                        zps = zp.tile([80, CH], fp32, tag="z", name="z")
                        st["zps"] = zps
                    zps = st["zps"]
                    first = (j == 0)
                    for k, ip in enumerate((2 * j, 2 * j + 1)):
                        for ii in range(2):
                            i = 2 * ip + ii
                            c0 = 128 * (i - 4 * j)
                            MM(zps[0:65, c0:CH],
                               vb_4d[:, h, i, :],
                               pts[(h, ip, j)][:, 512 * ii + c0:512 * ii + 512],
                               start=(first and k == 0 and ii == 0),
                               stop=(k == 1 and ii == 1))
                return go

            def evac_block(h, j, st, srec):
                def go():
                    c = CH * (h % 2)
                    nc.vector.tensor_copy(srec[0:1, c:c + CH],
                                          st["zps"][64:65, :])
                    if c:
                        # both heads' sums in one custom-DVE recip call
                        nc.vector.reciprocal_approx_fast(srec[:], srec[:])
                        rd = rdp.tile([1, 2 * CH], fp32, tag="rd", name="rd")
                        nc.gpsimd.dma_start(rd[:], srec[:])
                        st["rd"] = rd
                return go

            def bcast_block(h, j, st, st1):
                def go():
                    rd = st1["rd"]
                    c = CH * (h % 2)
                    rbc = rbp.tile([64, CH], fp32, tag="rbc", name="rbc")
                    nc.gpsimd.dma_start(
                        rbc[:], rd[0:1, c:c + CH].to_broadcast((64, CH)))
                    st["rbc"] = rbc
                return go

            def mul_block(h, j, st):
                def go():
                    nc.vector.tensor_mul(
                        znT[h // 2][j][64 * (h % 2):64 * (h % 2) + 64, :],
                        st["zps"][0:64, :], st["rbc"][:])
                return go

            def c_mm(m, n, st):
                def go():
                    ps = op.tile([128, CH], fp32, tag="o", name="o")
                    for t in range(2):
                        MM(ps[:],
                           znT[t][m // 4][:, 128 * (m % 4):128 * (m % 4) + 128],
                           woT_s[t][:, CH * n:CH * n + CH],
                           start=(t == 0), stop=(t == 1))
                    st["ps"] = ps
                return go

            def c_evac(m, n, st):
                def go():
                    ot = ob.tile([128, CH], bf16, tag="osb", name="osb")
                    nc.vector.tensor_copy(ot[:], st["ps"][:])
                    nc.sync.dma_start(
                        out_d[128 * m:128 * m + 128, CH * n:CH * n + CH],
                        ot[:])
                return go

            pending = []
            reserve = []

            def pop(n=1):
                for _ in range(n):
                    if pending:
                        pending.pop(0)[1]()

            def queue_group(j, hp, dl):
                heads = (2 * hp, 2 * hp + 1)
                sts = {h: {} for h in heads}
                srec = smp.tile([1, 2 * CH], fp32, tag="srec", name="srec")
                pending.extend([(dl, pv_off(h, j, sts[h])) for h in heads])
                pending.extend([(dl, pv_diag(h, j, sts[h])) for h in heads])
                pending.extend([(dl, evac_block(h, j, sts[h], srec))
                                for h in heads])
                pending.extend([(dl, bcast_block(h, j, sts[h],
                                                 sts[heads[1]]))
                                for h in heads])
                pending.extend([(dl, mul_block(h, j, sts[h])) for h in heads])

            def phase_c_blocks(j, dl):
                cst = [[{} for n in range(2)] for mm_ in range(4)]
                blocks = []
                for mi in range(0, 4, 2):
                    for n in range(2):
                        blocks.append((dl, c_mm(4 * j + mi, n, cst[mi][n])))
                        blocks.append((dl, c_mm(4 * j + mi + 1, n,
                                                cst[mi + 1][n])))
                        blocks.append((dl, c_evac(4 * j + mi, n, cst[mi][n])))
                        blocks.append((dl, c_evac(4 * j + mi + 1, n,
                                                  cst[mi + 1][n])))
                return blocks

            def run_spairs(j, hp):
                heads = (2 * hp, 2 * hp + 1)
                # diagonal pairs first: their exps start the group's
                # serial chain as early as possible
                ips = list(range(2 * j + 2))
                ips = ips[-2:] + ips[:-2]
                for ip in ips:
                    for h in heads:
                        s_pair(h, ip, j)
                    pop(2 if len(pending) >= 8 else 1)

            # j=0 and j=1 groups are tiny (2 and 4 score-pairs) but carry the
            # same ~8us serial chain as big groups; emit all four groups'
            # s_pairs back-to-back so their chains overlap each other.
            for hp in range(2):
                run_spairs(0, hp)
                queue_group(0, hp, 2)
            for hp in range(2):
                run_spairs(1, hp)
                queue_group(1, hp, 3)
            pending += phase_c_blocks(0, 3)
            pending += phase_c_blocks(1, 3)
            for j in (2, 3):
                for hp in range(2):
                    run_spairs(j, hp)
                    while pending:
                        pop()
                    if j == 3 and hp == 1:
                        # ready j=2 filler ahead of the final group's
                        # chain-dependent blocks in the in-order PE queue
                        pending.extend(reserve)
                    queue_group(j, hp, 2 * j + hp + 2)
                    if hp == 1:
                        blocks = phase_c_blocks(j, 2 * j + hp + 2)
                        if j == 2:
                            # hold back half of j=2's phase C as tail filler
                            pending += blocks[:8]
                            reserve = blocks[8:]
                        else:
                            pending += blocks
            while pending:
                pop()

    nc.finalize()
    return nc


_NC_CACHE = {}


def _bias2(b):
    return np.ascontiguousarray(b.reshape(2, 128).T.astype(np.float32))


def kernel(x, attn_mask, wq_w, wq_b, wk_w, wk_b, wv_w, wv_b, wo_w, wo_b):
    import ml_dtypes
    from concourse.bass_utils import run_bass_kernel_spmd

    bf = ml_dtypes.bfloat16
    x = np.asarray(x, np.float32)
    attn_mask = np.asarray(attn_mask)
    wq_w = np.asarray(wq_w, np.float32); wq_b = np.asarray(wq_b, np.float32)
    wk_w = np.asarray(wk_w, np.float32); wk_b = np.asarray(wk_b, np.float32)
    wv_w = np.asarray(wv_w, np.float32); wv_b = np.asarray(wv_b, np.float32)
    wo_w = np.asarray(wo_w, np.float32); wo_b = np.asarray(wo_b, np.float32)

    CT, SST = _rope_tables()
    RT = np.zeros((128, 4, 2, CH), np.float32)
    for j in range(4):
        RT[:, j, 0, :] = CT[:, CH * j:CH * j + CH]
        RT[:, j, 1, :] = SST[:, CH * j:CH * j + CH]
    RT = RT.reshape(128, -1).astype(bf)
    # -1e30 on the strict lower triangle (keys > queries), added to scores
    # in PSUM before exp
    tri01 = np.tril(np.full((128, 128), -1e30, np.float32), -1)

    def dmajor(wt, width):
        # [1024, width] -> [128, 8, width] with d-tiles contiguous per row
        return np.ascontiguousarray(
            wt.reshape(8, 128, width).transpose(1, 0, 2).reshape(128, -1))

    in_maps = []
    for c in range(NC):
        b, g = c // 4, c % 4
        fs = slice(F * g, F * g + F)
        wv = wv_w[fs]
        vb = wv_b[fs]
        wvT = np.zeros((DIM, 272), np.float32)
        vb1 = np.zeros((1, 272), np.float32)
        for h in range(HPC):
            wvT[:, 68 * h:68 * h + 64] = wv[64 * h:64 * h + 64].T
            vb1[0, 68 * h + 64] = 1.0
            vb1[0, 68 * h:68 * h + 64] = vb[64 * h:64 * h + 64]
        # x: [2048, 1024] -> feature-major [1024, 2048] -> [128, j, d, t]
        xb = np.ascontiguousarray(x[b].T).astype(np.float32)
        xb = xb.reshape(8, 128, 4, CH).transpose(1, 2, 0, 3).reshape(128, -1)
        woT = wo_w[:, fs].T  # [256, 1024]
        woT = np.ascontiguousarray(
            woT.reshape(2, 128, DIM).transpose(1, 0, 2).reshape(128, -1))
        in_maps.append({
            "xT": np.ascontiguousarray(xb).astype(bf),
            "wqT": dmajor(np.ascontiguousarray(wq_w[fs].T) / np.float32(8.0),
                          F).astype(bf),
            "wkT": dmajor(np.ascontiguousarray(wk_w[fs].T), F).astype(bf),
            "wvT": dmajor(wvT, 272).astype(bf),
            "woT": woT.astype(bf),
            "qb": _bias2(wq_b[fs] / np.float32(8.0)),
            "kb": _bias2(wk_b[fs]),
            "vb": vb1,
            # col0: 8*mask for fp8 v; col1: mask/4 for bf16 v (so both
            # paths contribute p*v/4 and p/4 to psum; the ratio is exact)
            "maskv": np.ascontiguousarray(np.stack(
                [attn_mask[b].astype(np.float32) * 8.0,
                 attn_mask[b].astype(np.float32) * 0.25],
                1).reshape(16, 128, 2).transpose(1, 0, 2).reshape(128, 32)),
            "RT": RT, "tri": tri01,
        })

    if "nc" not in _NC_CACHE:
        _NC_CACHE["nc"] = _build_program()
    res = run_bass_kernel_spmd(_NC_CACHE["nc"], in_maps, core_ids=list(range(NC)))
    globals()["LAST_RESULTS"] = res

    out = np.zeros((B, S, DIM), np.float32)
    for c in range(NC):
        out[c // 4] += np.asarray(res.results[c]["outp"], np.float32)
    out += wo_b[None, None, :]
    return out


if __name__ == "__main__":
    rng = np.random.default_rng(0)
    ins = {
        "x": rng.standard_normal((B, S, DIM), np.float32),
        "attn_mask": np.ones((B, S), bool),
    }
    for n in ["wq", "wk", "wv", "wo"]:
        ins[n + "_w"] = (rng.standard_normal((DIM, DIM), np.float32) / 32.0)
        ins[n + "_b"] = rng.standard_normal(DIM, np.float32) * 0.01
    o = kernel(**ins)
    print("ran", o.shape, o.dtype)
